# revision 10
# baseline (speedup 1.0000x reference)
"""Trainium2 Bass kernel for DigitConvolutionalModel.

Math: logits = relu(conv2d_valid(x.reshape(B,28,28), conv_w).reshape(B,676) @ W1 + b1) @ W2 + b2

Optimizations:
  1. The valid 3x3 conv is linear in x, so it folds into W1 on host:
     feat @ W1 == x @ (C @ W1) where C[784,676] scatters conv_w taps.
     The device then runs two dense matmuls per batch shard:
       h = relu(x @ W1eff + b1);  logits = h @ W2 + b2
  2. Sharding: batch 32768 split as 8 x 4096 across cores. The 784-pixel
     contraction is chunked 6x128 + 16 so the bulk of x moves in
     full-128-partition DMAs (measured ~20% faster per byte than
     112-partition transfers); the 16-pixel remainder (131KB) rides the
     gpsimd SWDGE ring. Host packs x partition-major ([128, nblk, 6,
     512] bf16) so every DMA is fully contiguous per partition.
  3. x, W1eff, h, W2 are bf16 on device (fp32 PSUM accumulation): halves
     DMA bytes AND streams the PE at 1 cycle/row. Measured end-to-end
     rel err ~3.6e-3 against the fp32 reference.
  4. DMA schedule (the kernel is PE-bound once supply is right): the
     sync HWDGE ring carries w1-main then all of x in consumption order
     (block 0 in 2-chunk pieces for progressive start, then one ~790KB
     DMA per block -- the measured single-ring sweet spot). Everything
     lands ~10us before the PE needs it; queues never cross-starve
     (heavily loaded rings starve light ones, so ordering is explicit).
  5. The PE starts real work ~8.5us (right after the framework preamble
     + first transfers) with 2 dummy warm-up matmuls to open the HAM
     clock-ramp window; the stream then runs gap-free so the clock
     stays at 2.4GHz. The last block interleaves its two output halves
     per k-chunk to shorten the relu->MM2->bias->out tail.

Device kernel (per core, per 512-column block):
  - MM1: hT[256,512] = W1eff.T @ xT, 6 k-chunks of 128 + 1 of 16
  - ACT: relu(hT + b1) PSUM->SBUF, output bf16
  - MM2 (pipelined one block behind): logitsT[10,512] over 2 chunks
  - DVE: + b2 (per-partition scalar add) PSUM->SBUF logitsT buffer
  - per-block DMA of logitsT slice; host transposes back to [B, 10]
"""
import ml_dtypes
import numpy as np

import concourse.bacc as bacc
import concourse.mybir as mybir
from concourse.tile import TileContext
from concourse.bass_utils import run_bass_kernel_spmd

B = 32768
IMG = 28
KSZ = 3
OUT_HW = IMG - KSZ + 1  # 26
FEAT = OUT_HW * OUT_HW  # 676
PIX = IMG * IMG  # 784
HID = 256
NCLS = 10
N_CORES = 8
BC = B // N_CORES  # 4096 rows per core
NBLK_COLS = 512  # batch columns per pipeline block (1 PSUM bank of fp32)
KCH = 128  # main contraction chunk: 784 = 6*128 + 16
NKC = 6
KREM = PIX - NKC * KCH  # 16
# wb blob layout (per partition, fp32 words): b1[2] | b2
WB_B1 = 0          # [128, 2]
WB_B2 = 2          # [128, 1] (only partitions 0..9 used)
WB_W = 3

f32 = mybir.dt.float32
bf16 = mybir.dt.bfloat16
AF = mybir.ActivationFunctionType

X_DT = bf16
W_DT = bf16
H_DT = bf16

N_WARMUP = 2

_CACHE = {}


def _build(bc=BC):
    """Build the single-core Bass program (SPMD across 8 cores)."""
    nblk = bc // NBLK_COLS
    nc = bacc.Bacc()
    # x main part, host-packed partition-major: [partition p, block,
    # k-chunk c, batch col]; pixel index = c*128 + p for c < 6
    xm = nc.declare_dram_parameter("xm", [KCH, nblk, NKC, NBLK_COLS], X_DT, isOutput=False)
    # x remainder: pixels 768..783 -> [16, nblk, 512]
    xr = nc.declare_dram_parameter("xr", [KREM, nblk, NBLK_COLS], X_DT, isOutput=False)
    # w1 main [128, 6, 256] (chunk-major per partition) + remainder [16, 256]
    w1m = nc.declare_dram_parameter("w1m", [KCH, NKC, HID], W_DT, isOutput=False)
    w1r = nc.declare_dram_parameter("w1r", [KREM, HID], W_DT, isOutput=False)
    # w2 blob: [128, 2, 10] (chunk-major per partition)
    w2 = nc.declare_dram_parameter("w2b", [128, 2, NCLS], W_DT, isOutput=False)
    # small-weights blob: [128, WB_W] fp32, see WB_* offsets
    wb = nc.declare_dram_parameter("wb", [128, WB_W], f32, isOutput=False)
    # output is logitsT [10, bc]; host transposes back
    out = nc.declare_dram_parameter("out", [NCLS, bc], f32, isOutput=True)

    with TileContext(nc) as tc:
        with (
            tc.tile_pool(name="weights", bufs=1) as wpool,
            tc.tile_pool(name="h_sb", bufs=4) as hpool,
            tc.tile_pool(name="h_ps", bufs=4, space="PSUM") as hps,
            tc.tile_pool(name="log_ps", bufs=2, space="PSUM") as logps,
        ):
            # ---- sync HWDGE ring, strict consumption order: w1 chunk 0,
            # x block-0 chunk 0, rest of w1, x block-0 chunks 1-5 singly
            # (progressive start at the slow small-packet phase), blocks
            # 1-2 in half-block pieces (smooth catch-up), rest whole ----
            w1m_sb = wpool.tile([KCH, NKC, HID], W_DT)
            xm_sb = wpool.tile([KCH, nblk, NKC, NBLK_COLS], X_DT)
            last_blk = nblk - 1
            nc.sync.dma_start(out=w1m_sb[:, 0:1, :], in_=w1m[:, 0:1, :])
            nc.sync.dma_start(out=xm_sb[:, 0, 0:1, :], in_=xm[:, 0, 0:1, :])
            nc.sync.dma_start(out=w1m_sb[:, 1:NKC, :], in_=w1m[:, 1:NKC, :])
            for c in range(1, NKC):
                nc.sync.dma_start(out=xm_sb[:, 0, c : c + 1, :], in_=xm[:, 0, c : c + 1, :])
            half = NKC // 2
            for b in range(1, nblk):
                if b <= 2 and nblk > 4:
                    nc.sync.dma_start(
                        out=xm_sb[:, b, 0:half, :], in_=xm[:, b, 0:half, :]
                    )
                    nc.sync.dma_start(
                        out=xm_sb[:, b, half:NKC, :], in_=xm[:, b, half:NKC, :]
                    )
                else:
                    nc.sync.dma_start(out=xm_sb[:, b : b + 1], in_=xm[:, b : b + 1])

            # ---- tiny weights on the scalar ring (needed later; safe
            # even if the loaded sync ring starves this one for a while)
            wb_sb = wpool.tile([128, WB_W], f32)
            nc.scalar.dma_start(out=wb_sb[:], in_=wb[:])
            w2_sb = wpool.tile([128, 2, NCLS], W_DT)
            nc.scalar.dma_start(out=w2_sb[:], in_=w2[:])
            w1r_sb = wpool.tile([KREM, HID], W_DT)
            nc.scalar.dma_start(out=w1r_sb[:], in_=w1r[:])

            # ---- x remainder on the gpsimd SWDGE ring, split so piece k
            # lands before block k's 7th k-chunk needs it ----
            xr_sb = wpool.tile([KREM, nblk, NBLK_COLS], X_DT)
            xr_splits = [(0, 1), (1, 3), (3, nblk)] if nblk > 3 else [(0, nblk)]
            for b0_, b1_ in xr_splits:
                nc.gpsimd.dma_start(out=xr_sb[:, b0_:b1_], in_=xr[:, b0_:b1_])

            b1_sb = wb_sb[:, WB_B1:WB_B2]
            b2_sb = wb_sb[:NCLS, WB_B2:WB_W]
            # all blocks' logitsT accumulate here; per-block drain
            log_all = wpool.tile([NCLS, bc], f32)

            # tiny warm-up: start the HAM activity window while block-0
            # DMAs land (2 matmuls on a zeroed tile)
            warm_a = wpool.tile([KCH, 128], X_DT)
            warm_b = wpool.tile([KCH, NBLK_COLS], X_DT)
            nc.vector.memset(warm_a[:], 0.0)
            nc.vector.memset(warm_b[:], 0.0)
            warm_ps = hps.tile([128, NBLK_COLS], f32, tag="h_ps")
            for _ in range(N_WARMUP):
                nc.tensor.matmul(
                    warm_ps[:], warm_a[:], warm_b[:], start=True, stop=True,
                    skip_group_check=True,
                )

            # ---- main pipeline over 512-column blocks ----
            # MM2 for block n is emitted during block n+1's MM1 so the PE
            # never waits on the relu round-trip.
            pending = None  # (hs, b0) awaiting MM2

            def emit_mm2(hs, b0, last=False):
                log_ps = logps.tile([NCLS, NBLK_COLS], f32)
                for mc in range(2):
                    nc.tensor.matmul(
                        log_ps[:],
                        w2_sb[:, mc, :],
                        hs[mc][:],
                        start=(mc == 0),
                        stop=(mc == 1),
                    )
                nc.vector.tensor_scalar_add(
                    out=log_all[:, b0 : b0 + NBLK_COLS],
                    in0=log_ps[:],
                    scalar1=b2_sb[:, 0:1],
                )
                eng = nc.scalar if last else nc.gpsimd
                eng.dma_start(
                    out=out[:, b0 : b0 + NBLK_COLS],
                    in_=log_all[:, b0 : b0 + NBLK_COLS],
                )

            def mm1(h_ps, blk, mc, kc):
                if kc < NKC:
                    nc.tensor.matmul(
                        h_ps[:],
                        w1m_sb[:, kc, mc * 128 : (mc + 1) * 128],
                        xm_sb[:, blk, kc, :],
                        start=(kc == 0),
                        stop=False,
                    )
                else:
                    nc.tensor.matmul(
                        h_ps[:],
                        w1r_sb[:, mc * 128 : (mc + 1) * 128],
                        xr_sb[:, blk, :],
                        start=False,
                        stop=True,
                    )

            # every block interleaves mc0/mc1 per k-chunk: each landed
            # chunk yields two matmuls of work (halves the early
            # consumption rate -> no supply stalls, HAM stays busy), and
            # both h halves close right after the last chunk. MM2 of the
            # previous block is emitted after chunk 3, by which time its
            # relu round-trip has completed (no PE wait).
            for blk in range(nblk):
                b0 = blk * NBLK_COLS
                h_ps2 = [
                    hps.tile([128, NBLK_COLS], f32, name=f"h_ps_{blk}_{mc}", tag="h_ps")
                    for mc in range(2)
                ]
                for kc in range(NKC + 1):
                    for mc in range(2):
                        mm1(h_ps2[mc], blk, mc, kc)
                    if kc == 3 and pending is not None:
                        emit_mm2(*pending)
                        pending = None
                hs = []
                for mc in range(2):
                    h_sb = hpool.tile(
                        [128, NBLK_COLS], H_DT, tag="h", name=f"h_{blk}_{mc}"
                    )
                    nc.scalar.activation(
                        h_sb[:], h_ps2[mc][:], AF.Relu, bias=b1_sb[:, mc : mc + 1]
                    )
                    hs.append(h_sb)
                pending = (hs, b0)

            emit_mm2(*pending, last=True)

    nc.compile()
    return nc


def _fold_conv_into_w1(conv_w, W1):
    """W1eff[784, 256] such that x @ W1eff == conv(x) flattened @ W1."""
    conv_w = np.asarray(conv_w, dtype=np.float64)
    W1 = np.asarray(W1, dtype=np.float64)
    C = np.zeros((IMG, IMG, OUT_HW, OUT_HW), dtype=np.float64)
    oi = np.arange(OUT_HW)[:, None]
    oj = np.arange(OUT_HW)[None, :]
    for ki in range(KSZ):
        for kj in range(KSZ):
            C[oi + ki, oj + kj, oi, oj] = conv_w[ki, kj]
    W1eff = C.reshape(PIX, FEAT) @ W1
    return np.ascontiguousarray(W1eff, dtype=np.float32)


def _pack_weights(w1e, b1, W2, b2):
    np_wdt = mybir.dt.np(W_DT)
    w1m = np.ascontiguousarray(
        w1e[: NKC * KCH].reshape(NKC, KCH, HID).transpose(1, 0, 2).astype(np_wdt)
    )
    w1r = np.ascontiguousarray(w1e[NKC * KCH :].astype(np_wdt))
    w2b = np.ascontiguousarray(
        W2.reshape(2, 128, NCLS).transpose(1, 0, 2).astype(np_wdt)
    )
    wb = np.zeros((128, WB_W), dtype=np.float32)
    wb[:, WB_B1:WB_B2] = b1.reshape(2, 128).T
    wb[:NCLS, WB_B2] = b2
    return w1m, w1r, w2b, wb


def kernel(x, conv_w, W1, b1, W2, b2, _bc=BC, _trace=False):
    x = np.asarray(x, dtype=np.float32)
    w1e = _fold_conv_into_w1(conv_w, W1)
    b1 = np.asarray(b1, dtype=np.float32)
    W2 = np.asarray(W2, dtype=np.float32)
    b2 = np.asarray(b2, dtype=np.float32)
    w1m, w1r, w2b, wb = _pack_weights(w1e, b1, W2, b2)

    n_cores = x.shape[0] // _bc
    if _bc not in _CACHE:
        _CACHE[_bc] = _build(_bc)
    nc = _CACHE[_bc]

    nblk = _bc // NBLK_COLS
    np_xdt = mybir.dt.np(X_DT)
    in_maps = []
    for c in range(n_cores):
        xc = x[c * _bc : (c + 1) * _bc]
        in_maps.append(
            {
                # [bc, 768] -> [nblk, 512, 6, 128] -> [128, nblk, 6, 512]
                "xm": np.ascontiguousarray(
                    xc[:, : NKC * KCH]
                    .reshape(nblk, NBLK_COLS, NKC, KCH)
                    .transpose(3, 0, 2, 1)
                    .astype(np_xdt)
                ),
                # [bc, 16] -> [16, nblk, 512]
                "xr": np.ascontiguousarray(
                    xc[:, NKC * KCH :]
                    .reshape(nblk, NBLK_COLS, KREM)
                    .transpose(2, 0, 1)
                    .astype(np_xdt)
                ),
                "w1m": w1m,
                "w1r": w1r,
                "w2b": w2b,
                "wb": wb,
            }
        )
    res = run_bass_kernel_spmd(
        nc, in_maps, core_ids=list(range(n_cores)), trace=_trace
    )
    # device layout logitsT [10, bc] -> [bc, 10]
    out = np.concatenate(
        [np.ascontiguousarray(res.results[c]["out"].T) for c in range(n_cores)],
        axis=0,
    )
    if _trace:
        return out, res
    return out


# revision 11
# speedup vs baseline: 1.0344x; 1.0344x over previous
"""Trainium2 Bass kernel for DigitConvolutionalModel.

Math: logits = relu(conv2d_valid(x.reshape(B,28,28), conv_w).reshape(B,676) @ W1 + b1) @ W2 + b2

Optimizations:
  1. The valid 3x3 conv is linear in x, so it folds into W1 on host:
     feat @ W1 == x @ (C @ W1) where C[784,676] scatters conv_w taps.
     The device then runs two dense matmuls per batch shard:
       h = relu(x @ W1eff + b1);  logits = h @ W2 + b2
  2. Sharding: batch 32768 split as 8 x 4096 across cores. The 784-pixel
     contraction is chunked 6x128 + 16 so the bulk of x moves in
     full-128-partition DMAs (measured ~20% faster per byte than
     112-partition transfers); the 16-pixel remainder (131KB) rides the
     gpsimd SWDGE ring. Host packs x partition-major ([128, nblk, 6,
     512] bf16) so every DMA is fully contiguous per partition.
  3. x, W1eff, h, W2 are bf16 on device (fp32 PSUM accumulation): halves
     DMA bytes AND streams the PE at 1 cycle/row. Measured end-to-end
     rel err ~3.6e-3 against the fp32 reference.
  4. DMA schedule (the kernel is PE-bound once supply is right): the
     sync HWDGE ring carries w1-main then all of x in consumption order
     (block 0 in 2-chunk pieces for progressive start, then one ~790KB
     DMA per block -- the measured single-ring sweet spot). Everything
     lands ~10us before the PE needs it; queues never cross-starve
     (heavily loaded rings starve light ones, so ordering is explicit).
  5. The PE starts real work ~8.5us (right after the framework preamble
     + first transfers) with 2 dummy warm-up matmuls to open the HAM
     clock-ramp window; the stream then runs gap-free so the clock
     stays at 2.4GHz. The last block interleaves its two output halves
     per k-chunk to shorten the relu->MM2->bias->out tail.

Device kernel (per core, per 512-column block):
  - MM1: hT[256,512] = W1eff.T @ xT, 6 k-chunks of 128 + 1 of 16
  - ACT: relu(hT + b1) PSUM->SBUF, output bf16
  - MM2 (pipelined one block behind): logitsT[10,512] over 2 chunks
  - DVE: + b2 (per-partition scalar add) PSUM->SBUF logitsT buffer
  - per-block DMA of logitsT slice; host transposes back to [B, 10]
"""
import ml_dtypes
import numpy as np

import concourse.bacc as bacc
import concourse.mybir as mybir
from concourse.tile import TileContext
from concourse.bass_utils import run_bass_kernel_spmd

B = 32768
IMG = 28
KSZ = 3
OUT_HW = IMG - KSZ + 1  # 26
FEAT = OUT_HW * OUT_HW  # 676
PIX = IMG * IMG  # 784
HID = 256
NCLS = 10
N_CORES = 8
BC = B // N_CORES  # 4096 rows per core
NBLK_COLS = 512  # batch columns per pipeline block (1 PSUM bank of fp32)
KCH = 128  # main contraction chunk: 784 = 6*128 + 16
NKC = 6
KREM = PIX - NKC * KCH  # 16
# wb blob layout (per partition, fp32 words): b1[2] | b2
WB_B1 = 0          # [128, 2]
WB_B2 = 2          # [128, 1] (only partitions 0..9 used)
WB_W = 3

f32 = mybir.dt.float32
bf16 = mybir.dt.bfloat16
AF = mybir.ActivationFunctionType

X_DT = bf16
W_DT = bf16
H_DT = bf16

N_WARMUP = 2

_CACHE = {}


def _build(bc=BC):
    """Build the single-core Bass program (SPMD across 8 cores)."""
    nblk = bc // NBLK_COLS
    nc = bacc.Bacc()
    # x main part, host-packed partition-major: [partition p, block,
    # k-chunk c, batch col]; pixel index = c*128 + p for c < 6
    xm = nc.declare_dram_parameter("xm", [KCH, nblk, NKC, NBLK_COLS], X_DT, isOutput=False)
    # x remainder: pixels 768..783 -> [16, nblk, 512]
    xr = nc.declare_dram_parameter("xr", [KREM, nblk, NBLK_COLS], X_DT, isOutput=False)
    # w1 main [128, 6, 256] (chunk-major per partition) + remainder [16, 256]
    w1m = nc.declare_dram_parameter("w1m", [KCH, NKC, HID], W_DT, isOutput=False)
    w1r = nc.declare_dram_parameter("w1r", [KREM, HID], W_DT, isOutput=False)
    # w2 blob: [128, 2, 10] (chunk-major per partition)
    w2 = nc.declare_dram_parameter("w2b", [128, 2, NCLS], W_DT, isOutput=False)
    # small-weights blob: [128, WB_W] fp32, see WB_* offsets
    wb = nc.declare_dram_parameter("wb", [128, WB_W], f32, isOutput=False)
    # output is logitsT [10, bc]; host transposes back
    out = nc.declare_dram_parameter("out", [NCLS, bc], f32, isOutput=True)

    with TileContext(nc) as tc:
        with (
            tc.tile_pool(name="weights", bufs=1) as wpool,
            tc.tile_pool(name="h_sb", bufs=4) as hpool,
            tc.tile_pool(name="h_ps", bufs=4, space="PSUM") as hps,
            tc.tile_pool(name="log_ps", bufs=2, space="PSUM") as logps,
        ):
            # ---- sync HWDGE ring: all of x-main in consumption order.
            # DMA issue occupies its queue ~650ns each, so the head uses
            # few, medium pieces; each engine's first issue starts right
            # after the preamble ----
            w1m_sb = wpool.tile([KCH, NKC, HID], W_DT)
            xm_sb = wpool.tile([KCH, nblk, NKC, NBLK_COLS], X_DT)
            last_blk = nblk - 1
            half = NKC // 2
            nc.sync.dma_start(out=xm_sb[:, 0, 0:half, :], in_=xm[:, 0, 0:half, :])
            nc.sync.dma_start(out=xm_sb[:, 0, half:NKC, :], in_=xm[:, 0, half:NKC, :])
            for b in range(1, nblk):
                nc.sync.dma_start(out=xm_sb[:, b : b + 1], in_=xm[:, b : b + 1])

            # ---- weights on the scalar ring, in consumption order; the
            # first piece (w1 chunk 0) gates the PE start ----
            w1r_sb = wpool.tile([KREM, HID], W_DT)
            wb_sb = wpool.tile([128, WB_W], f32)
            w2_sb = wpool.tile([128, 2, NCLS], W_DT)
            nc.scalar.dma_start(out=w1m_sb[:, 0:1, :], in_=w1m[:, 0:1, :])
            nc.scalar.dma_start(out=w1m_sb[:, 1:NKC, :], in_=w1m[:, 1:NKC, :])
            nc.scalar.dma_start(out=wb_sb[:], in_=wb[:])
            nc.scalar.dma_start(out=w1r_sb[:], in_=w1r[:])
            nc.scalar.dma_start(out=w2_sb[:], in_=w2[:])

            # ---- x remainder on the gpsimd SWDGE ring, split so piece k
            # lands before block k's 7th k-chunk needs it ----
            xr_sb = wpool.tile([KREM, nblk, NBLK_COLS], X_DT)
            xr_splits = [(0, 1), (1, 3), (3, nblk)] if nblk > 3 else [(0, nblk)]
            for b0_, b1_ in xr_splits:
                nc.gpsimd.dma_start(out=xr_sb[:, b0_:b1_], in_=xr[:, b0_:b1_])

            b1_sb = wb_sb[:, WB_B1:WB_B2]
            b2_sb = wb_sb[:NCLS, WB_B2:WB_W]
            # all blocks' logitsT accumulate here; per-block drain
            log_all = wpool.tile([NCLS, bc], f32)

            # tiny warm-up: start the HAM activity window while block-0
            # DMAs land (2 matmuls on a zeroed tile)
            warm_a = wpool.tile([KCH, 128], X_DT)
            warm_b = wpool.tile([KCH, NBLK_COLS], X_DT)
            nc.vector.memset(warm_a[:], 0.0)
            nc.vector.memset(warm_b[:], 0.0)
            warm_ps = hps.tile([128, NBLK_COLS], f32, tag="h_ps")
            for _ in range(N_WARMUP):
                nc.tensor.matmul(
                    warm_ps[:], warm_a[:], warm_b[:], start=True, stop=True,
                    skip_group_check=True,
                )

            # ---- main pipeline over 512-column blocks ----
            # MM2 for block n is emitted during block n+1's MM1 so the PE
            # never waits on the relu round-trip.
            pending = None  # (hs, b0) awaiting MM2

            def emit_mm2(hs, b0, last=False):
                log_ps = logps.tile([NCLS, NBLK_COLS], f32)
                for mc in range(2):
                    nc.tensor.matmul(
                        log_ps[:],
                        w2_sb[:, mc, :],
                        hs[mc][:],
                        start=(mc == 0),
                        stop=(mc == 1),
                    )
                nc.vector.tensor_scalar_add(
                    out=log_all[:, b0 : b0 + NBLK_COLS],
                    in0=log_ps[:],
                    scalar1=b2_sb[:, 0:1],
                )
                eng = nc.scalar if last else nc.gpsimd
                eng.dma_start(
                    out=out[:, b0 : b0 + NBLK_COLS],
                    in_=log_all[:, b0 : b0 + NBLK_COLS],
                )

            def mm1(h_ps, blk, mc, kc):
                if kc < NKC:
                    nc.tensor.matmul(
                        h_ps[:],
                        w1m_sb[:, kc, mc * 128 : (mc + 1) * 128],
                        xm_sb[:, blk, kc, :],
                        start=(kc == 0),
                        stop=False,
                    )
                else:
                    nc.tensor.matmul(
                        h_ps[:],
                        w1r_sb[:, mc * 128 : (mc + 1) * 128],
                        xr_sb[:, blk, :],
                        start=False,
                        stop=True,
                    )

            # every block interleaves mc0/mc1 per k-chunk: each landed
            # chunk yields two matmuls of work (halves the early
            # consumption rate -> no supply stalls, HAM stays busy), and
            # both h halves close right after the last chunk. MM2 of the
            # previous block is emitted after chunk 3, by which time its
            # relu round-trip has completed (no PE wait).
            for blk in range(nblk):
                b0 = blk * NBLK_COLS
                h_ps2 = [
                    hps.tile([128, NBLK_COLS], f32, name=f"h_ps_{blk}_{mc}", tag="h_ps")
                    for mc in range(2)
                ]
                for kc in range(NKC + 1):
                    for mc in range(2):
                        mm1(h_ps2[mc], blk, mc, kc)
                    if kc == 3 and pending is not None:
                        emit_mm2(*pending)
                        pending = None
                hs = []
                for mc in range(2):
                    h_sb = hpool.tile(
                        [128, NBLK_COLS], H_DT, tag="h", name=f"h_{blk}_{mc}"
                    )
                    nc.scalar.activation(
                        h_sb[:], h_ps2[mc][:], AF.Relu, bias=b1_sb[:, mc : mc + 1]
                    )
                    hs.append(h_sb)
                pending = (hs, b0)

            emit_mm2(*pending, last=True)

    nc.compile()
    return nc


def _fold_conv_into_w1(conv_w, W1):
    """W1eff[784, 256] such that x @ W1eff == conv(x) flattened @ W1."""
    conv_w = np.asarray(conv_w, dtype=np.float64)
    W1 = np.asarray(W1, dtype=np.float64)
    C = np.zeros((IMG, IMG, OUT_HW, OUT_HW), dtype=np.float64)
    oi = np.arange(OUT_HW)[:, None]
    oj = np.arange(OUT_HW)[None, :]
    for ki in range(KSZ):
        for kj in range(KSZ):
            C[oi + ki, oj + kj, oi, oj] = conv_w[ki, kj]
    W1eff = C.reshape(PIX, FEAT) @ W1
    return np.ascontiguousarray(W1eff, dtype=np.float32)


def _pack_weights(w1e, b1, W2, b2):
    np_wdt = mybir.dt.np(W_DT)
    w1m = np.ascontiguousarray(
        w1e[: NKC * KCH].reshape(NKC, KCH, HID).transpose(1, 0, 2).astype(np_wdt)
    )
    w1r = np.ascontiguousarray(w1e[NKC * KCH :].astype(np_wdt))
    w2b = np.ascontiguousarray(
        W2.reshape(2, 128, NCLS).transpose(1, 0, 2).astype(np_wdt)
    )
    wb = np.zeros((128, WB_W), dtype=np.float32)
    wb[:, WB_B1:WB_B2] = b1.reshape(2, 128).T
    wb[:NCLS, WB_B2] = b2
    return w1m, w1r, w2b, wb


def kernel(x, conv_w, W1, b1, W2, b2, _bc=BC, _trace=False):
    x = np.asarray(x, dtype=np.float32)
    w1e = _fold_conv_into_w1(conv_w, W1)
    b1 = np.asarray(b1, dtype=np.float32)
    W2 = np.asarray(W2, dtype=np.float32)
    b2 = np.asarray(b2, dtype=np.float32)
    w1m, w1r, w2b, wb = _pack_weights(w1e, b1, W2, b2)

    n_cores = x.shape[0] // _bc
    if _bc not in _CACHE:
        _CACHE[_bc] = _build(_bc)
    nc = _CACHE[_bc]

    nblk = _bc // NBLK_COLS
    np_xdt = mybir.dt.np(X_DT)
    in_maps = []
    for c in range(n_cores):
        xc = x[c * _bc : (c + 1) * _bc]
        in_maps.append(
            {
                # [bc, 768] -> [nblk, 512, 6, 128] -> [128, nblk, 6, 512]
                "xm": np.ascontiguousarray(
                    xc[:, : NKC * KCH]
                    .reshape(nblk, NBLK_COLS, NKC, KCH)
                    .transpose(3, 0, 2, 1)
                    .astype(np_xdt)
                ),
                # [bc, 16] -> [16, nblk, 512]
                "xr": np.ascontiguousarray(
                    xc[:, NKC * KCH :]
                    .reshape(nblk, NBLK_COLS, KREM)
                    .transpose(2, 0, 1)
                    .astype(np_xdt)
                ),
                "w1m": w1m,
                "w1r": w1r,
                "w2b": w2b,
                "wb": wb,
            }
        )
    res = run_bass_kernel_spmd(
        nc, in_maps, core_ids=list(range(n_cores)), trace=_trace
    )
    # device layout logitsT [10, bc] -> [bc, 10]
    out = np.concatenate(
        [np.ascontiguousarray(res.results[c]["out"].T) for c in range(n_cores)],
        axis=0,
    )
    if _trace:
        return out, res
    return out
